# revision 15
# baseline (speedup 1.0000x reference)
"""VisionZip text-aware token-selection kernel for Trainium2 (Bass/Tile).

Contract: kernel(**inputs) takes FULL inputs (B=32) and returns the FULL
output [32, 65, 1024]. Internally: pure data-parallel over 8 NeuronCores
(4 samples each).

Algorithm (per sample, all on device):
  score = 0.5*z(sum_h attn[h,0,1:]) + 0.5*z(cos(metric[1:], text))
  top-54 patches (+CLS) -> dominant mask m over 577 tokens (rank trick:
  rank_i = #{j: s_j > s_i} with s_0 = 1e30 sentinel; m = rank < 55)
  cums = cumsum(m) (upper-triangular ones matmul)
  p = i - cums  (position among remaining tokens)
  targets: remaining tokens with p in {0,52,...,468}
  merge tokens: remaining non-targets; assigned to argmax_r <mn_i, Tn_r>
  Output rows = C @ hidden where C[65, 577]:
    rows 0..54: one-hot at the r-th selected token (ascending)
    rows 55+r : one-hot at target r plus (1/count_r) over its merge tokens
"""
import numpy as np

import sys
if '/opt/trn_rl_repo' not in sys.path:
    sys.path.insert(0, '/opt/trn_rl_repo')

import concourse.bacc as bacc
import concourse.tile as tile
from concourse import mybir
from concourse.bass_utils import run_bass_kernel_spmd

F32 = mybir.dt.float32
N_CORES = 8
BC = 4                      # samples per core
L = 577                     # tokens (incl CLS)
D = 1024
CK = 64
NH = 16
DOM = 54                    # dominant patches
NSEL = DOM + 1              # + CLS
CTX = 10
STEP = 52                   # (577-1-54) // 10
OUT_T = NSEL + CTX          # 65 output tokens
CHUNKS = [(0, 128), (128, 128), (256, 128), (384, 128), (512, 65)]
LPAD = 640


def _consts():
    c = {}
    c["c_ones1"] = np.ones((1, 128), np.float32)
    oh = np.zeros((BC * NH, BC), np.float32)
    for s in range(BC):
        oh[s * NH:(s + 1) * NH, s] = 1.0
    c["c_oh64"] = oh
    c["c_iden"] = np.eye(128, dtype=np.float32)
    c["c_ones128"] = np.ones((128, 128), np.float32)
    ut = (np.arange(128)[:, None] <= np.arange(128)[None, :]).astype(np.float32)
    c["c_ut128"] = ut
    c["c_onescol"] = np.ones((128, 1), np.float32)
    c["c_iota55"] = np.broadcast_to(
        (np.arange(NSEL) + 1.0).astype(np.float32), (128, NSEL)).copy()
    iota52 = np.zeros((128, BC * CTX), np.float32)
    for s in range(BC):
        iota52[:, s * CTX:(s + 1) * CTX] = -STEP * np.arange(CTX, dtype=np.float32)
    c["c_iota52"] = iota52      # compare against pn = cums - i  (pn == -52r)
    ii = np.zeros((128, 5), np.float32)
    for ci, (off, _) in enumerate(CHUNKS):
        ii[:, ci] = off + np.arange(128)
    c["c_iotaI"] = ii
    selbc = np.zeros((BC, BC * 128), np.float32)
    for s in range(BC):
        selbc[s, s * 128:(s + 1) * 128] = 1.0
    c["c_selbc"] = selbc        # lhsT slice [BC,128] broadcasts row s to 128 parts
    return c


def build_nc(stage=99):
    nc = bacc.Bacc("TRN2", target_bir_lowering=False, debug=False)

    attn_d = nc.declare_dram_parameter("attn_row", [BC * NH, L], F32, isOutput=False)
    hidden_d = nc.declare_dram_parameter("hidden", [BC, L, D], F32, isOutput=False)
    metric_d = nc.declare_dram_parameter("metric", [BC, L, CK], F32, isOutput=False)
    text_d = nc.declare_dram_parameter("text", [BC, CK], F32, isOutput=False)
    cshapes = {k: v.shape for k, v in _consts().items()}
    cdram = {k: nc.declare_dram_parameter(k, list(sh), F32, isOutput=False)
             for k, sh in cshapes.items()}
    out_d = nc.declare_dram_parameter("out", [BC, OUT_T, D], F32, isOutput=True)

    with tile.TileContext(nc) as tc:
        with (
            tc.tile_pool(name="persist", bufs=1) as pp,
            tc.tile_pool(name="hidpool", bufs=1) as hp,
            tc.tile_pool(name="scratch", bufs=2) as sp,
            tc.tile_pool(name="cpool", bufs=6) as cp,
            tc.tile_pool(name="ps_misc", bufs=3, space="PSUM") as ps_misc,
            tc.tile_pool(name="ps_bcast", bufs=2, space="PSUM") as ps_bcast,
            tc.tile_pool(name="ps_out", bufs=3, space="PSUM") as ps_out,
        ):
            pools = (pp, hp, sp, cp, ps_misc, ps_bcast, ps_out)
            _body(nc, stage, pools, attn_d, hidden_d, metric_d, text_d,
                  cdram, cshapes, out_d)
    nc.compile()
    return nc


def _body(nc, stage, pools, attn_d, hidden_d, metric_d, text_d,
          cdram, cshapes, out_d):
    pp, hp, sp, cp, ps_misc, ps_bcast, ps_out = pools
    V = nc.vector
    A = nc.scalar
    T = nc.tensor
    DMA = nc.sync

    def dump(n):
        d = sp.tile([BC, 512], F32, tag="dump")
        V.memset(d[:], float(n))
        DMA.dma_start(out_d[:, 0, 0:512], d[:])

    # ---- constant + small input DMAs ----
    csb = {}
    for k, sh in cshapes.items():
        t = pp.tile(list(sh), F32, tag=k)
        DMA.dma_start(t[:], cdram[k][:])
        csb[k] = t
    attn_sb = pp.tile([BC * NH, L], F32, tag="attn_sb")
    DMA.dma_start(attn_sb[:], attn_d[:])
    text_sb = pp.tile([BC, CK], F32, tag="text_sb")
    DMA.dma_start(text_sb[:], text_d[:])
    mt = []
    for ci, (off, k) in enumerate(CHUNKS):
        t = pp.tile([128, BC, CK], F32, tag=f"mt{ci}")
        DMA.dma_start(t[0:k, :, :],
                      metric_d[:, off:off + k, :].rearrange("s l c -> l s c"))
        mt.append(t)

    # ---- hidden DMAs (big; stream in the background) ----
    hid = []      # hid[s][ci] -> [128, 1024] tile (chunk 4: 65 rows valid)
    for s in range(BC):
        row = []
        for ci, (off, k) in enumerate(CHUNKS):
            t = hp.tile([128, D], F32, tag=f"h{s}_{ci}")
            DMA.dma_start(t[0:k, :], hidden_d[s, off:off + k, :])
            row.append(t)
        hid.append(row)

    if stage <= 1:
        return dump(1)

    # ---- text_n ----
    tsc = sp.tile([BC, CK], F32, tag="tsc")
    tss = pp.tile([BC, 1], F32, tag="tss")
    V.tensor_mul(tsc[:], text_sb[:], text_sb[:])
    V.tensor_reduce(tss[:], tsc[:], axis=mybir.AxisListType.X,
                    op=mybir.AluOpType.add)
    tst = pp.tile([BC, 1], F32, tag="tst")
    A.activation(tst[:], tss[:], mybir.ActivationFunctionType.Sqrt)
    trc = pp.tile([BC, 1], F32, tag="trc")
    V.reciprocal(trc[:], tst[:])
    textn = pp.tile([BC, CK], F32, tag="textn")
    V.tensor_scalar_mul(textn[:], text_sb[:], trc[:])

    # textb: [128, (s,c)] broadcast of text_n along partitions
    tb_ps = ps_misc.tile([128, BC * CK], F32, tag="ps")
    for s in range(BC):
        T.matmul(tb_ps[:, s * CK:(s + 1) * CK],
                 csb["c_selbc"][:, s * 128:(s + 1) * 128],
                 textn[:, :], start=True, stop=True)
    textb = pp.tile([128, BC, CK], F32, tag="textb")
    V.tensor_copy(textb[:].rearrange("p s c -> p (s c)"), tb_ps[:, :])

    # ---- Sd row via matmul: out[s, i] = sum_h attn[(s,h), i] ----
    sd_ps1 = ps_bcast.tile([BC, 512], F32, tag="psb")
    sd_ps2 = ps_misc.tile([BC, L - 512], F32, tag="ps")
    T.matmul(sd_ps1[:, :], csb["c_oh64"][:, :], attn_sb[:, 0:512],
             start=True, stop=True)
    T.matmul(sd_ps2[:, :], csb["c_oh64"][:, :], attn_sb[:, 512:L],
             start=True, stop=True)
    sd_row = pp.tile([BC, LPAD], F32, tag="sd_row")
    V.tensor_copy(sd_row[:, 0:512], sd_ps1[:, :])
    V.tensor_copy(sd_row[:, 512:L], sd_ps2[:, :])

    # ---- metric norms, mn, cos, dot ----
    mn = []
    rnorm_all = pp.tile([128, 5, BC], F32, tag="rnorm_all")
    cosc = pp.tile([128, 5, BC], F32, tag="cosc")
    for ci, (off, k) in enumerate(CHUNKS):
        sq = sp.tile([128, BC, CK], F32, tag="sq")
        V.tensor_mul(sq[0:k], mt[ci][0:k], mt[ci][0:k])
        ssq = sp.tile([128, BC], F32, tag="ssq")
        V.tensor_reduce(ssq[0:k], sq[0:k], axis=mybir.AxisListType.X,
                        op=mybir.AluOpType.add)
        srt = sp.tile([128, BC], F32, tag="srt")
        A.activation(srt[0:k], ssq[0:k], mybir.ActivationFunctionType.Sqrt)
        V.reciprocal(rnorm_all[0:k, ci, :], srt[0:k])
        mnc = pp.tile([128, BC, CK], F32, tag=f"mn{ci}")
        for s in range(BC):
            V.tensor_scalar_mul(mnc[0:k, s, :], mt[ci][0:k, s, :],
                                rnorm_all[0:k, ci, s:s + 1])
        mn.append(mnc)
        # dot with text_n
        dq = sp.tile([128, BC, CK], F32, tag="dq")
        V.tensor_mul(dq[0:k], mt[ci][0:k], textb[0:k])
        dsum = sp.tile([128, BC], F32, tag="dsum")
        V.tensor_reduce(dsum[0:k], dq[0:k], axis=mybir.AxisListType.X,
                        op=mybir.AluOpType.add)
        V.tensor_mul(cosc[0:k, ci, :], dsum[0:k], rnorm_all[0:k, ci, :])

    # cos -> row layout [BC, L]
    cos_row = pp.tile([BC, LPAD], F32, tag="cos_row")
    for ci, (off, k) in enumerate(CHUNKS):
        cps = ps_misc.tile([BC, 128], F32, tag="ps")
        T.transpose(cps[:, 0:k], cosc[0:k, ci, :], csb["c_iden"][0:k, 0:k])
        V.tensor_copy(cos_row[:, off:off + k], cps[:, 0:k])

    if stage <= 2:
        return dump(2)

    # ---- z-scores -> score_row ----
    score_row = pp.tile([BC, LPAD], F32, tag="score_row")

    def zscore_into(row, xm_tag, invh_out):
        ssum = sp.tile([BC, 1], F32, tag=xm_tag + "_sum")
        V.tensor_reduce(ssum[:], row[:, 1:L], axis=mybir.AxisListType.X,
                        op=mybir.AluOpType.add)
        mean = sp.tile([BC, 1], F32, tag=xm_tag + "_mean")
        V.tensor_scalar_mul(mean[:], ssum[:], 1.0 / (L - 1))
        xm = pp.tile([BC, L - 1], F32, tag=xm_tag)
        V.tensor_scalar(xm[:], row[:, 1:L], mean[:], None,
                        op0=mybir.AluOpType.subtract)
        scr = sp.tile([BC, L - 1], F32, tag=xm_tag + "_scr")
        ssq2 = sp.tile([BC, 1], F32, tag=xm_tag + "_ssq")
        V.tensor_mul(scr[:], xm[:], xm[:])
        V.tensor_reduce(ssq2[:], scr[:], axis=mybir.AxisListType.X,
                        op=mybir.AluOpType.add)
        std = sp.tile([BC, 1], F32, tag=xm_tag + "_std")
        A.activation(std[:], ssq2[:], mybir.ActivationFunctionType.Sqrt,
                     scale=1.0 / (L - 2))
        den = sp.tile([BC, 1], F32, tag=xm_tag + "_den")
        V.tensor_scalar_add(den[:], std[:], 1e-6)
        inv = sp.tile([BC, 1], F32, tag=xm_tag + "_inv")
        V.reciprocal(inv[:], den[:])
        V.tensor_scalar_mul(invh_out[:], inv[:], 0.5)
        return xm

    invh_sd = sp.tile([BC, 1], F32, tag="invh_sd")
    xm_sd = zscore_into(sd_row, "xm_sd", invh_sd)
    invh_cos = sp.tile([BC, 1], F32, tag="invh_cos")
    xm_cos = zscore_into(cos_row, "xm_cos", invh_cos)
    V.tensor_scalar_mul(score_row[:, 1:L], xm_sd[:], invh_sd[:])
    V.scalar_tensor_tensor(out=score_row[:, 1:L], in0=xm_cos[:],
                           scalar=invh_cos[:], in1=score_row[:, 1:L],
                           op0=mybir.AluOpType.mult,
                           op1=mybir.AluOpType.add)
    V.memset(score_row[:, 0:1], 1.0e30)

    if stage <= 3:
        return dump(3)

    # ---- scoreT ----
    scoreT = pp.tile([128, 5, BC], F32, tag="scoreT")
    for ci, (off, k) in enumerate(CHUNKS):
        sps = ps_misc.tile([128, BC], F32, tag="ps")
        T.transpose(sps[0:k, :], score_row[:, off:off + k],
                    csb["c_iden"][0:BC, 0:BC])
        V.tensor_copy(scoreT[0:k, ci, :], sps[0:k, :])

    # ---- rank ----
    rank = pp.tile([128, 5, BC], F32, tag="rank")
    V.memset(rank[:], 1.0e9)
    for s in range(BC):
        bc_ps1 = ps_bcast.tile([128, 512], F32, tag="psb")
        T.matmul(bc_ps1[:, :], csb["c_selbc"][:, s * 128:(s + 1) * 128],
                 score_row[:, 0:512], start=True, stop=True)
        bc_ps2 = ps_misc.tile([128, L - 512], F32, tag="ps")
        T.matmul(bc_ps2[:, :], csb["c_selbc"][:, s * 128:(s + 1) * 128],
                 score_row[:, 512:L], start=True, stop=True)
        bcs = sp.tile([128, LPAD], F32, tag="bcs")
        V.tensor_copy(bcs[:, 0:512], bc_ps1[:, :])
        V.tensor_copy(bcs[:, 512:L], bc_ps2[:, :])
        for ci, (off, k) in enumerate(CHUNKS):
            g = sp.tile([128, LPAD], F32, tag="g")
            V.tensor_scalar(g[0:k, 0:L], bcs[0:k, 0:L],
                            scoreT[0:k, ci, s:s + 1], 0.0,
                            op0=mybir.AluOpType.is_gt,
                            op1=mybir.AluOpType.add,
                            accum_out=rank[0:k, ci, s:s + 1])

    if stage <= 4:
        return dump(4)

    # ---- m, cums, pn ----
    msk = pp.tile([128, 5, BC], F32, tag="msk")
    V.tensor_scalar(msk[:].rearrange("p c s -> p (c s)"),
                    rank[:].rearrange("p c s -> p (c s)"),
                    float(NSEL), None, op0=mybir.AluOpType.is_lt)
    cums = pp.tile([128, 5, BC], F32, tag="cums")
    V.memset(cums[:].rearrange("p c s -> p (c s)"), 0.0)
    for cm in range(5):
        cps2 = ps_misc.tile([128, BC], F32, tag="ps")
        for ck in range(cm + 1):
            lhs = csb["c_ut128"] if ck == cm else csb["c_ones128"]
            kk = CHUNKS[ck][1]
            T.matmul(cps2[0:CHUNKS[cm][1], :], lhs[0:kk, 0:CHUNKS[cm][1]],
                     msk[0:kk, ck, :], start=(ck == 0), stop=(ck == cm))
        V.tensor_copy(cums[0:CHUNKS[cm][1], cm, :], cps2[0:CHUNKS[cm][1], :])
    pn = pp.tile([128, 5, BC], F32, tag="pn")       # pn = cums - i
    for ci in range(5):
        V.tensor_scalar(pn[:, ci, :], cums[:, ci, :],
                        csb["c_iotaI"][:, ci:ci + 1], None,
                        op0=mybir.AluOpType.subtract)
    notm = pp.tile([128, 5, BC], F32, tag="notm")
    V.tensor_scalar(notm[:].rearrange("p c s -> p (c s)"),
                    msk[:].rearrange("p c s -> p (c s)"),
                    0.5, None, op0=mybir.AluOpType.is_lt)

    if stage <= 5:
        return dump(5)

    # ---- Itgt, is_mrg ----
    itgt = []
    ismrg = pp.tile([128, 5, BC], F32, tag="ismrg")
    V.memset(ismrg[:].rearrange("p c s -> p (c s)"), 0.0)
    for ci, (off, k) in enumerate(CHUNKS):
        it = pp.tile([128, BC, CTX], F32, tag=f"itgt{ci}")
        for s in range(BC):
            V.tensor_scalar(it[0:k, s, :],
                            csb["c_iota52"][0:k, s * CTX:(s + 1) * CTX],
                            pn[0:k, ci, s:s + 1], None,
                            op0=mybir.AluOpType.is_equal)
            V.tensor_scalar_mul(it[0:k, s, :], it[0:k, s, :],
                                notm[0:k, ci, s:s + 1])
        itgt.append(it)
        tany = sp.tile([128, BC], F32, tag="tany")
        V.tensor_reduce(tany[0:k], it[0:k], axis=mybir.AxisListType.X,
                        op=mybir.AluOpType.add)
        e = sp.tile([128, BC], F32, tag="e_mrg")
        V.tensor_mul(e[0:k], notm[0:k, ci, :], tany[0:k])
        V.tensor_sub(ismrg[0:k, ci, :], notm[0:k, ci, :], e[0:k])
    # chunk-4 rows 65.. (tokens i > 576) stay 0 from the memset above

    if stage <= 6:
        return dump(6)

    # ---- mnT (per sample) ----
    mnT = []
    for s in range(BC):
        t = pp.tile([CK, LPAD], F32, tag=f"mnT{s}")
        for ci, (off, k) in enumerate(CHUNKS):
            tps = ps_misc.tile([CK, 128], F32, tag="ps")
            T.transpose(tps[:, 0:k], mn[ci][0:k, s, :],
                        csb["c_iden"][0:k, 0:k])
            V.tensor_copy(t[:, off:off + k], tps[:, 0:k])
        mnT.append(t)

    # ---- Tn ----
    tn_sb = pp.tile([CK, BC, CTX], F32, tag="tn_sb")
    for s in range(BC):
        tn_ps = ps_misc.tile([CK, CTX], F32, tag="ps")
        for ci, (off, k) in enumerate(CHUNKS):
            T.matmul(tn_ps[:, :], mn[ci][0:k, s, :], itgt[ci][0:k, s, :],
                     start=(ci == 0), stop=(ci == 4))
        V.tensor_copy(tn_sb[:, s, :], tn_ps[:, :])

    # ---- sim, rowmax, eq, eqM ----
    eqm = []
    for ci, (off, k) in enumerate(CHUNKS):
        sim_sb = sp.tile([128, BC, CTX], F32, tag="sim_sb")
        for s in range(BC):
            sim_ps = ps_misc.tile([128, CTX], F32, tag="ps")
            T.matmul(sim_ps[0:k, :], mnT[s][:, off:off + k],
                     tn_sb[:, s, :], start=True, stop=True)
            V.tensor_copy(sim_sb[0:k, s, :], sim_ps[0:k, :])
        rmx = sp.tile([128, BC], F32, tag="rmx")
        V.tensor_reduce(rmx[0:k], sim_sb[0:k], axis=mybir.AxisListType.X,
                        op=mybir.AluOpType.max)
        em = pp.tile([128, BC, CTX], F32, tag=f"eqm{ci}")
        for s in range(BC):
            V.tensor_scalar(em[0:k, s, :], sim_sb[0:k, s, :],
                            rmx[0:k, s:s + 1], None,
                            op0=mybir.AluOpType.is_ge)
            V.tensor_scalar_mul(em[0:k, s, :], em[0:k, s, :],
                                ismrg[0:k, ci, s:s + 1])
        eqm.append(em)

    if stage <= 7:
        return dump(7)

    # ---- counts -> 1/max(counts,1) -> cb ----
    cnt_ps = ps_misc.tile([1, BC * CTX], F32, tag="ps")
    for ci, (off, k) in enumerate(CHUNKS):
        T.matmul(cnt_ps[:, :], csb["c_onescol"][0:k, :],
                 eqm[ci][0:k].rearrange("p s c -> p (s c)"),
                 start=(ci == 0), stop=(ci == 4))
    crow = sp.tile([1, BC * CTX], F32, tag="crow")
    V.tensor_scalar_max(crow[:], cnt_ps[:, :], 1.0)
    crec = pp.tile([1, BC * CTX], F32, tag="crec")
    V.reciprocal(crec[:], crow[:])
    cb_ps = ps_misc.tile([128, BC * CTX], F32, tag="ps")
    T.matmul(cb_ps[:, :], csb["c_ones1"][:, :], crec[:, :],
             start=True, stop=True)
    cb = pp.tile([128, BC, CTX], F32, tag="cb")
    V.tensor_copy(cb[:].rearrange("p s c -> p (s c)"), cb_ps[:, :])

    if stage <= 8:
        return dump(8)

    # ---- C build + big matmuls + out DMA ----
    for s in range(BC):
        cts = []
        for ci, (off, k) in enumerate(CHUNKS):
            ct = cp.tile([128, 80], F32, tag="C")
            V.tensor_scalar(ct[0:k, 0:NSEL], csb["c_iota55"][0:k, :],
                            cums[0:k, ci, s:s + 1], None,
                            op0=mybir.AluOpType.is_equal)
            V.tensor_scalar_mul(ct[0:k, 0:NSEL], ct[0:k, 0:NSEL],
                                msk[0:k, ci, s:s + 1])
            wct = sp.tile([128, CTX], F32, tag="wct")
            V.tensor_mul(wct[0:k, :], eqm[ci][0:k, s, :], cb[0:k, s, :])
            V.tensor_add(ct[0:k, NSEL:OUT_T], itgt[ci][0:k, s, :],
                         wct[0:k, :])
            cts.append(ct)
        for n2 in range(2):
            po = ps_out.tile([OUT_T, 512], F32, tag="po")
            for ci, (off, k) in enumerate(CHUNKS):
                T.matmul(po[:, :], cts[ci][0:k, 0:OUT_T],
                         hid[s][ci][0:k, n2 * 512:(n2 + 1) * 512],
                         start=(ci == 0), stop=(ci == 4))
            ob = sp.tile([OUT_T, 512], F32, tag="ob")
            V.tensor_copy(ob[:, :], po[:, :])
            DMA.dma_start(out_d[s, :, n2 * 512:(n2 + 1) * 512], ob[:, :])


_NC = None


def _get_nc():
    global _NC
    if _NC is None:
        _NC = build_nc()
    return _NC


def shard_inputs(attn_weights, hidden_states, metric, text_emb):
    """Host-side shard: slice the CLS attention row; split batch across cores."""
    B = attn_weights.shape[0]
    per = B // N_CORES
    attn_row = np.ascontiguousarray(attn_weights[:, :, 0, :])   # [B, 16, 577]
    consts = _consts()
    in_maps = []
    for c in range(N_CORES):
        sl = slice(c * per, (c + 1) * per)
        m = {
            "attn_row": np.ascontiguousarray(
                attn_row[sl].reshape(per * NH, L)).astype(np.float32),
            "hidden": np.ascontiguousarray(hidden_states[sl]).astype(np.float32),
            "metric": np.ascontiguousarray(metric[sl]).astype(np.float32),
            "text": np.ascontiguousarray(text_emb[sl]).astype(np.float32),
        }
        m.update(consts)
        in_maps.append(m)
    return in_maps


def kernel(attn_weights, hidden_states, metric, text_emb):
    nc = _get_nc()
    in_maps = shard_inputs(attn_weights, hidden_states, metric, text_emb)
    res = run_bass_kernel_spmd(nc, in_maps, core_ids=list(range(N_CORES)))
    out = np.concatenate([r["out"] for r in res.results], axis=0)
    return out.astype(np.float32)
